# revision 23
# baseline (speedup 1.0000x reference)
"""AttentionPool2d Trainium2 kernel, 8-core batch-data-parallel.

Math (reference returns only query position 0):
  xf = [mean, x.flat] + pos ; only q at position 0 matters.
  Host folds: xp = x + pos_sp (bf16), xf0 = mean_s(x) + pos0,
  u = (1/8) W_k_h^T (W_q xf0 + b_q)  (tiny: 64x1024x16, f32 on host).
  Device per batch b:
    lg[h, s] = sum_c u[c,h] xp[c,s]          (spatial logits)
    lg[h, 256] = u . posc                     (posc = pos0 - mean pos_sp)
    lg_mt = mean_s lg[:, :256] + lg[:, 256]   (mean-token logit, linearity)
    softmax (no max-sub; |logit| << 1); fold mean token into spatial
    weights: w'' = (ex_sp + ex_mt/256) / Z, alpha = ex_mt / Z
    y[c, h] = sum_s xp[c, s] w''[s, h] + posc[c] * alpha[h]
    a0 = blockdiag(W_v) y + b_v ;  outT = a0^T-proj via w_c (b_c on host)
"""
import sys
sys.path.insert(0, "/opt/trn_rl_repo")
import numpy as np
import ml_dtypes
from contextlib import ExitStack

from concourse import bacc, tile, mybir
import concourse.bass as bass
from concourse import masks
from concourse.bass_utils import run_bass_kernel_spmd

P = 128
B, C, S2, L = 64, 1024, 256, 257
NH, CHD = 16, 64
NCORE, BPC, CT = 8, 8, 8
F32R = mybir.dt.float32r
F32 = mybir.dt.float32
BF16 = mybir.dt.bfloat16
AF = mybir.ActivationFunctionType
X = mybir.AxisListType.X
bf16 = ml_dtypes.bfloat16


def _body(ctx: ExitStack, tc, d):
    nc = tc.nc
    const = ctx.enter_context(tc.tile_pool(name="const", bufs=1))
    xpool = ctx.enter_context(tc.tile_pool(name="xpool", bufs=1))
    wpool = ctx.enter_context(tc.tile_pool(name="wpool", bufs=1))
    work = ctx.enter_context(tc.tile_pool(name="work", bufs=1))
    psL = ctx.enter_context(tc.tile_pool(name="psL", bufs=2, space="PSUM"))
    psW = ctx.enter_context(tc.tile_pool(name="psW", bufs=1, space="PSUM"))
    psY = ctx.enter_context(tc.tile_pool(name="psY", bufs=1, space="PSUM"))
    psA = ctx.enter_context(tc.tile_pool(name="psA", bufs=1, space="PSUM"))
    psO = ctx.enter_context(tc.tile_pool(name="psO", bufs=1, space="PSUM"))

    # ---- input DMAs in consumption order ----
    # small tensors go via the Activation-engine DGE so the SP sequencer
    # streams the big x tensors back-to-back without stalls
    u = wpool.tile([P, CT, BPC, 16], BF16)          # (c-part, j, b, h)
    nc.scalar.dma_start(u[:], d["u"].ap())
    poscrow = wpool.tile([1, C], BF16)              # posc row (1, c)
    nc.scalar.dma_start(poscrow[:], d["poscrow"].ap())
    xns = []
    for g in range(4):                              # (c-part, 2b, j, 257)
        xn = xpool.tile([P, 2, CT, L], BF16, tag=f"xn{g}")
        nc.sync.dma_start(xn[:], d[f"xn{g}"].ap())
        xns.append(xn)
    xts = []
    for g in range(4):                              # (s-part, 2b, t, c)
        xt = xpool.tile([P, 2, 2, C], BF16, tag=f"xt{g}")
        nc.sync.dma_start(xt[:], d[f"xt{g}"].ap())
        xts.append(xt)
    wvt = wpool.tile([P, CT, C], BF16)              # W_v^T (c-part, j, v)
    nc.sync.dma_start(wvt[:], d["wvt"].ap())
    wcts = []
    for g in range(4):                              # w_c^T (v-part, 2r, o)
        wct = wpool.tile([P, 2, C], BF16, tag=f"wct{g}")
        nc.sync.dma_start(wct[:], d[f"wct{g}"].ap())
        wcts.append(wct)

    # full 128x128 identity for PE transposes
    identf = const.tile([P, P], F32)
    masks.make_identity(nc, identf[:])
    identb = const.tile([P, P], BF16)
    nc.vector.tensor_copy(identb[:], identf[:])

    # batch b -> (group g, slot q): matmul out base 32q (96 is not allowed)
    GROUPS = [(0, 1, 2), (3, 4, 5), (6, 7)]

    # ---- logits + softmax, batched 3-wide per (128, 257) PSUM tile ----
    # no max-subtraction: |logit| << 1 by construction
    # wtp[:, g, t, :] = transpose of group-g weights; cols are 32q+h (+garbage)
    wtp = psW.tile([P, 3, 2, P], BF16)
    aT = work.tile([1, 3, 4, 32], BF16)             # alpha rows, (g, q, 32blk)
    for g, grp in enumerate(GROUPS):
        lgA = psL.tile([P, L], F32, tag="lg")
        for q, b in enumerate(grp):
            xb = xns[b // 2][:, b % 2]              # (c-part, j, 257)
            for j in range(CT):
                nc.tensor.matmul(lgA[32 * q:32 * q + 16, :], u[:, j, b, :],
                                 xb[:, j, :],
                                 start=(j == 0), stop=(j == CT - 1))
        st = work.tile([P, 8], F32, tag=f"st{g}")
        nc.vector.reduce_sum(st[:, 0:1], lgA[:, 0:S2], axis=X)
        nc.vector.tensor_scalar_mul(st[:, 3:4], st[:, 0:1], 1.0 / S2)
        ex = work.tile([P, L], F32R, tag=f"ex{g}")
        nc.scalar.activation(ex[:, 0:S2], lgA[:, 0:S2], AF.Exp,
                             accum_out=st[:, 1:2])
        # mean-token logit = mean(spatial lg) + posc-term (lg col 256)
        nc.scalar.activation(ex[:, S2:L], lgA[:, S2:L], AF.Exp,
                             bias=st[:, 3:4], accum_out=st[:, 2:3])
        nc.vector.tensor_add(st[:, 4:5], st[:, 1:2], st[:, 2:3])
        nc.vector.reciprocal(st[:, 5:6], st[:, 4:5])
        nc.vector.tensor_scalar_mul(st[:, 6:7], ex[:, S2:L], 1.0 / S2)
        wsp = work.tile([P, S2], BF16, tag=f"ws{g}")
        nc.vector.tensor_scalar(wsp[:, :], ex[:, 0:S2], st[:, 6:7], st[:, 5:6],
                                op0=mybir.AluOpType.add,
                                op1=mybir.AluOpType.mult)
        alp = work.tile([P, 1], BF16, tag=f"al{g}")
        nc.vector.tensor_scalar(alp[:, :], ex[:, S2:L], st[:, 5:6], None,
                                op0=mybir.AluOpType.mult)
        # full-width transposes (offset-partition transposes fail at runtime)
        for t in range(2):
            nc.tensor.transpose(wtp[:, g, t, :], wsp[:, t * P:(t + 1) * P],
                                identb[:])
        # alpha (128,1) -> row (1,128) via tiny SBUF->SBUF DMA on gpsimd
        nc.gpsimd.dma_start(aT[0:1, g], alp[:, 0:1])

    wT = work.tile([P, 3, 2, P], BF16)
    nc.vector.tensor_copy(wT[:, 0], wtp[:, 0])
    nc.scalar.activation(wT[:, 1], wtp[:, 1], AF.Copy)
    nc.vector.tensor_copy(wT[:, 2], wtp[:, 2])

    # ---- y[c, (b,h)] = sum_s xp w'' + posc outer alpha ----
    # all start=True resets are emitted before any accumulation in a bank
    yps = psY.tile([P, CT, BPC, 16], F32)           # (c-part, j, b, h)
    for j in range(CT):
        for g, grp in enumerate(GROUPS):
            nq = len(grp)
            nc.tensor.matmul(yps[:, j, grp[0]:grp[0] + nq, :],
                             poscrow[0:1, j * P:(j + 1) * P],
                             aT[0:1, g, 0:nq, 0:16],
                             start=True, stop=False, skip_group_check=True)
    for b in range(BPC):
        g, q = (b // 3, b % 3) if b < 6 else (2, b - 6)
        xtb = xts[b // 2][:, b % 2]                 # (s-part, t, c)
        for j in range(CT):
            for t in range(2):
                nc.tensor.matmul(yps[:, j, b, :],
                                 xtb[:, t, j * P:(j + 1) * P],
                                 wT[:, g, t, 32 * q:32 * q + 16],
                                 start=False, stop=(t == 1),
                                 skip_group_check=True)
    yfin = work.tile([P, CT, BPC, 16], BF16)
    nc.vector.tensor_copy(yfin[:, 0:4], yps[:, 0:4])
    nc.scalar.activation(yfin[:, 4:8], yps[:, 4:8], AF.Copy)

    # ---- a0 = blockdiag(W_v) y  (b_v folded into host output add) ----
    a0ps = psA.tile([P, CT, 16], F32)               # (v-part, r, (b,hh))
    for r in range(CT):
        for j in range(CT):
            nc.tensor.matmul(a0ps[:, r, :], wvt[:, j, r * P:(r + 1) * P],
                             yfin[:, j, :, 2 * r:2 * r + 2],
                             start=(j == 0), stop=(j == CT - 1))
    a0sb = work.tile([P, CT, BPC], BF16)            # (v-part, r, b)
    nc.vector.tensor_copy(a0sb[0:64], a0ps[0:64, :, 0:16:2])
    nc.scalar.activation(a0sb[64:P], a0ps[64:P, :, 1:16:2], AF.Copy)

    # ---- outT[b, o] = sum_v a0[v, b] w_c[o, v]  (b_c added on host) ----
    outps = psO.tile([BPC, C], F32)
    for r in range(CT):
        wct = wcts[r // 2]
        for g in range(2):
            nc.tensor.matmul(outps[:, g * 512:(g + 1) * 512],
                             a0sb[:, r, :],
                             wct[:, r % 2, g * 512:(g + 1) * 512],
                             start=(r == 0), stop=(r == CT - 1))
    osb = work.tile([BPC, C], F32)
    nc.vector.tensor_copy(osb[:, 0:512], outps[:, 0:512])
    nc.scalar.activation(osb[:, 512:C], outps[:, 512:C], AF.Copy)
    nc.gpsimd.dma_start(d["out"].ap()[:, 0:512], osb[:, 0:512])
    nc.gpsimd.dma_start(d["out"].ap()[:, 512:C], osb[:, 512:C])


_CACHE = {}


def _get_nc():
    if "nc" in _CACHE:
        return _CACHE["nc"]
    nc = bacc.Bacc("TRN2", target_bir_lowering=False, debug=False,
                   num_devices=NCORE)
    d = {}
    d["u"] = nc.dram_tensor("u", [P, CT, BPC, 16], BF16, kind="ExternalInput")
    d["poscrow"] = nc.dram_tensor("poscrow", [1, C], BF16, kind="ExternalInput")
    for g in range(4):
        d[f"xn{g}"] = nc.dram_tensor(f"xn{g}", [P, 2, CT, L], BF16,
                                     kind="ExternalInput")
        d[f"xt{g}"] = nc.dram_tensor(f"xt{g}", [P, 2, 2, C], BF16,
                                     kind="ExternalInput")
    d["wvt"] = nc.dram_tensor("wvt", [P, CT, C], BF16, kind="ExternalInput")
    for g in range(4):
        d[f"wct{g}"] = nc.dram_tensor(f"wct{g}", [P, 2, C], BF16,
                                      kind="ExternalInput")
    d["out"] = nc.dram_tensor("out", [BPC, C], F32, kind="ExternalOutput")
    with tile.TileContext(nc) as tc, ExitStack() as ctx, \
            nc.allow_low_precision(reason="float32r tiles hold f32 bits"):
        _body(ctx, tc, d)
    nc.compile()
    _CACHE["nc"] = nc
    return nc


def _prep_maps(inputs):
    x = inputs["x"].reshape(B, C, S2).astype(np.float32)
    pos = inputs["pos_emb"].astype(np.float32)
    pos_sp = pos[:, 1:]
    posc = pos[:, 0] - pos_sp.mean(axis=1)
    wqkv = inputs["w_qkv"].astype(np.float32)
    Wq, Wk, Wv = wqkv[0:C], wqkv[C:2 * C], wqkv[2 * C:3 * C]
    bq = inputs["b_qkv"][0:C].astype(np.float32)
    bvv = inputs["b_qkv"][2 * C:3 * C].astype(np.float32)
    wc = inputs["w_c"].astype(np.float32)

    # host fold: u = (1/8) W_k_h^T (W_q xf0 + b_q)   (f32, tiny)
    xf0 = x.mean(axis=2) + pos[:, 0][None]
    q0 = xf0 @ Wq.T + bq[None]
    uf = np.einsum("bhq,hqc->bch", q0.reshape(B, NH, CHD),
                   Wk.reshape(NH, CHD, C)) * 0.125

    xp16 = (x + pos_sp[None]).astype(bf16)
    posc16 = posc.astype(bf16)
    xn_all = np.empty((B, CT, P, L), bf16)
    xn_all[..., :S2] = xp16.reshape(B, CT, P, S2)
    xn_all[..., S2] = posc16.reshape(CT, P)[None]
    xt_all = np.ascontiguousarray(
        xp16.reshape(B, C, 2, P).transpose(3, 0, 2, 1))      # (p, b, t, c)
    u_all = np.ascontiguousarray(
        uf.astype(bf16).reshape(B, CT, P, 16).transpose(2, 1, 0, 3))
    wvt = np.ascontiguousarray(
        Wv.T.reshape(CT, P, C).transpose(1, 0, 2)).astype(bf16)
    wctf = np.ascontiguousarray(
        wc.T.reshape(CT, P, C).transpose(1, 0, 2)).astype(bf16)
    shared = {
        "poscrow": np.ascontiguousarray(posc16[None, :]),
        "wvt": wvt,
    }
    for g in range(4):
        shared[f"wct{g}"] = np.ascontiguousarray(wctf[:, 2 * g:2 * g + 2])
    maps = []
    for c in range(NCORE):
        m = dict(shared)
        b0 = c * BPC
        m["u"] = np.ascontiguousarray(u_all[:, :, b0:b0 + BPC])
        for g in range(4):
            bb = b0 + 2 * g
            m[f"xn{g}"] = np.ascontiguousarray(
                xn_all[bb:bb + 2].transpose(2, 0, 1, 3))
            m[f"xt{g}"] = np.ascontiguousarray(xt_all[:, bb:bb + 2])
        maps.append(m)
    return maps


def kernel(**inputs) -> np.ndarray:
    nc = _get_nc()
    maps = _prep_maps(inputs)
    res = run_bass_kernel_spmd(nc, maps, list(range(NCORE)))
    # host-folded constants: b_c plus w_c @ b_v (b_v is batch-independent)
    bvv = inputs["b_qkv"][2 * C:3 * C].astype(np.float32)
    corr = inputs["w_c"].astype(np.float32) @ bvv + inputs["b_c"].astype(np.float32)
    out = np.empty((B, C), np.float32)
    for c in range(NCORE):
        out[c * BPC:(c + 1) * BPC] = res.results[c]["out"]
    return out + corr[None, :]


if __name__ == "__main__":
    rng = np.random.default_rng(0)
    ins = {
        "x": rng.standard_normal((B, C, 16, 16), dtype=np.float32),
        "pos_emb": rng.standard_normal((C, L), dtype=np.float32) / 32,
        "w_qkv": rng.standard_normal((3 * C, C), dtype=np.float32) / 32,
        "b_qkv": rng.standard_normal((3 * C,), dtype=np.float32) * 0.1,
        "w_c": rng.standard_normal((C, C), dtype=np.float32) / 32,
        "b_c": rng.standard_normal((C,), dtype=np.float32) * 0.1,
    }
    o = kernel(**ins)
    print("out", o.shape, o.dtype, float(np.abs(o).mean()))
